# revision 4
# baseline (speedup 1.0000x reference)
"""Gaussian distance loss kernel for 8 Trainium2 NeuronCores.

reference math (per term):
    f[i,j,d] = exp(-0.5*(mu1[i,d]-mu2[j,d])^2 / vsum) / sqrt(vsum),
    vsum = v1[i,d]+v2[j,d];  out = mean(f_aa) + mean(f_bb) - 2*mean(f_ab).

Identity used:  f = E * R * (sqrt(pi)/2)  with
    R = 1/sqrt(vsum)            (Abs_reciprocal_sqrt activation)
    E = DErf(dm*R/sqrt(2)) = (2/sqrt(pi)) * exp(-0.5*(dm*R)^2)

so each (row i, term) needs only FIVE ops on [128(d), 1024(j)] tiles:
    ACT  R  = AbsRsqrt(cv + rv_i)     (bias AP fuses the broadcast add)
    DVE  dm = cm - rm_i               (tensor_scalar, 4x mode, 327ns)
    DVE  t  = dm * R   (in-place)     (tensor_tensor, 2x mode, 594ns)
    ACT  E  = DErf(t / sqrt(2))       (in-place on t)
    DVE  stt: acc[:,col] = sum_j E*R  (scalar_tensor_tensor + accum_out;
         tensor_tensor_reduce would fit but crashes the device runtime)

This balances the two busy engines (ACT ~0.80ms, DVE ~0.79ms per pass
per core) instead of the previous DVE-bound wide-block design (DVE
1.44ms, ACT 0.99ms, 2.10ms total in CoreSim).

Abs_reciprocal_sqrt and Derivative_Erf live in DIFFERENT activation
table sets, so rows are processed in chunks of 32: all R ops of a chunk
back-to-back (one table load), then all E ops (one load) -- 24 table
switches per pass (~31us) instead of one per instruction.

Sharding: rows i split across 8 cores (128 rows per core per term);
each core holds the full transposed operands [128(d), 1024(j)].
Columns are bf16 (rounding bias cancels between vaa+vbb and -2*vab);
row scalars stay f32. Accumulation is f32 on-chip, f64 on host.
"""

import sys

for _p in ("/opt/trn_rl_repo", "/root/.axon_site/_ro/trn_rl_repo"):
    if _p not in sys.path:
        sys.path.append(_p)

import math

import ml_dtypes
import numpy as np

N = 1024
D = 128
NCORES = 8
ROWS = N // NCORES  # 128 rows per core per term
CHUNK = 32  # rows per activation-table phase
NCHUNKS = ROWS // CHUNK

O_MA, O_MB, O_VA, O_VB = 0, N, 2 * N, 3 * N
PACKED_BF_W = 4 * N
# f32 row-parameter tensor: rm_a, rv_a, rm_b, rv_b, each [D, ROWS]
O_RMA, O_RVA, O_RMB, O_RVB = 0, ROWS, 2 * ROWS, 3 * ROWS
PACKED_F_W = 4 * ROWS

NACC = 3 * ROWS  # one f32 accumulator column per (term, row)


def build_program(repeat=1):
    import concourse.bacc as bacc
    import concourse.mybir as mybir
    import concourse.tile as tile
    from concourse.alu_op_type import AluOpType

    f32 = mybir.dt.float32
    bf16 = mybir.dt.bfloat16
    Act = mybir.ActivationFunctionType
    Alu = AluOpType

    nc = bacc.Bacc("TRN2", target_bir_lowering=False, debug=False)
    inp_d = nc.dram_tensor("inp", [D, PACKED_BF_W], bf16, kind="ExternalInput")
    inpf_d = nc.dram_tensor("inpf", [D, PACKED_F_W], f32, kind="ExternalInput")
    acc_out = nc.dram_tensor("acc", [D, NACC], f32, kind="ExternalOutput")

    ISQRT2 = 1.0 / math.sqrt(2.0)

    with tile.TileContext(nc) as tc:
        with (
            tc.tile_pool(name="inputs", bufs=1) as inp,
            tc.tile_pool(name="inputsf", bufs=1) as inpf,
            tc.tile_pool(name="accp", bufs=1) as accp,
            tc.tile_pool(name="Rp", bufs=CHUNK + 4) as Rp,
            tc.tile_pool(name="Tp", bufs=CHUNK + 4) as Tp,
        ):
            big = inp.tile([D, PACKED_BF_W], bf16, tag="big")
            nc.sync.dma_start(big[:], inp_d[:])
            bigf = inpf.tile([D, PACKED_F_W], f32, tag="bigf")
            nc.sync.dma_start(bigf[:], inpf_d[:])

            cm_a = big[:, O_MA : O_MA + N]
            cm_b = big[:, O_MB : O_MB + N]
            cv_a = big[:, O_VA : O_VA + N]
            cv_b = big[:, O_VB : O_VB + N]
            rm_a = bigf[:, O_RMA : O_RMA + ROWS]
            rv_a = bigf[:, O_RVA : O_RVA + ROWS]
            rm_b = bigf[:, O_RMB : O_RMB + ROWS]
            rv_b = bigf[:, O_RVB : O_RVB + ROWS]

            acc = accp.tile([D, NACC], f32, tag="acc")

            terms = [
                (cm_a, cv_a, rm_a, rv_a),  # vaa
                (cm_b, cv_b, rm_a, rv_a),  # vab
                (cm_b, cv_b, rm_b, rv_b),  # vbb
            ] * repeat

            for ti, (cm, cv, rm, rv) in enumerate(terms):
                t0 = (ti % 3) * ROWS  # repeats overwrite the same columns
                for ch in range(NCHUNKS):
                    i0 = ch * CHUNK
                    Rts = []
                    Tts = []
                    # phase A: R = AbsRsqrt(cv + rv_i) back-to-back on ACT;
                    # DVE computes dm and t = dm*R as each R lands.
                    for r in range(CHUNK):
                        i = i0 + r
                        R = Rp.tile([D, N], bf16, tag="R")
                        nc.scalar.activation(
                            R[:], cv, Act.Abs_reciprocal_sqrt,
                            bias=rv[:, i : i + 1],
                        )
                        T = Tp.tile([D, N], bf16, tag="T")
                        nc.vector.tensor_scalar(
                            T[:], cm, rm[:, i : i + 1], None, Alu.subtract
                        )
                        nc.vector.tensor_tensor(T[:], T[:], R[:], Alu.mult)
                        Rts.append(R)
                        Tts.append(T)
                    # phase B: E = DErf(t/sqrt(2)) back-to-back on ACT;
                    # DVE reduces sum_j E*R as each E lands.
                    for r in range(CHUNK):
                        i = i0 + r
                        R, T = Rts[r], Tts[r]
                        nc.scalar.activation(
                            T[:], T[:], Act.Derivative_Erf, scale=ISQRT2
                        )
                        col = t0 + i
                        nc.vector.scalar_tensor_tensor(
                            T[:], T[:], 1.0, R[:], Alu.bypass, Alu.mult,
                            accum_out=acc[:, col : col + 1],
                        )

            nc.sync.dma_start(acc_out[:], acc[:])

    nc.compile()
    return nc


_PROGRAM_CACHE = {}


def _get_program(repeat=1):
    if repeat not in _PROGRAM_CACHE:
        _PROGRAM_CACHE[repeat] = build_program(repeat)
    return _PROGRAM_CACHE[repeat]


def pack_inputs(mu_a, logvar_a, mu_b, logvar_b):
    ma_t = np.ascontiguousarray(np.asarray(mu_a).T.astype(np.float32))
    mb_t = np.ascontiguousarray(np.asarray(mu_b).T.astype(np.float32))
    va_t = np.exp(np.asarray(logvar_a).T.astype(np.float32))
    vb_t = np.exp(np.asarray(logvar_b).T.astype(np.float32))
    packed_bf = np.empty((D, PACKED_BF_W), dtype=np.float32)
    packed_bf[:, O_MA : O_MA + N] = ma_t
    packed_bf[:, O_MB : O_MB + N] = mb_t
    packed_bf[:, O_VA : O_VA + N] = va_t
    packed_bf[:, O_VB : O_VB + N] = vb_t
    packed_bf = packed_bf.astype(ml_dtypes.bfloat16)
    in_maps = []
    for c in range(NCORES):
        r0, r1 = c * ROWS, (c + 1) * ROWS
        packed_f = np.empty((D, PACKED_F_W), dtype=np.float32)
        packed_f[:, O_RMA : O_RMA + ROWS] = ma_t[:, r0:r1]
        packed_f[:, O_RVA : O_RVA + ROWS] = va_t[:, r0:r1]
        packed_f[:, O_RMB : O_RMB + ROWS] = mb_t[:, r0:r1]
        packed_f[:, O_RVB : O_RVB + ROWS] = vb_t[:, r0:r1]
        in_maps.append({"inp": packed_bf, "inpf": packed_f})
    return in_maps


def run_device(mu_a, logvar_a, mu_b, logvar_b, trace=False, repeat=1):
    from concourse.bass_utils import run_bass_kernel_spmd

    nc = _get_program(repeat)
    in_maps = pack_inputs(mu_a, logvar_a, mu_b, logvar_b)
    return run_bass_kernel_spmd(nc, in_maps, list(range(NCORES)), trace=trace)


def reduce_host(results):
    saa = sab = sbb = 0.0
    for r in results:
        acc = np.asarray(r["acc"], dtype=np.float64)
        saa += acc[:, 0:ROWS].sum()
        sab += acc[:, ROWS : 2 * ROWS].sum()
        sbb += acc[:, 2 * ROWS : 3 * ROWS].sum()
    denom = float(N) * N * D
    scale = math.sqrt(math.pi) / 2.0
    return np.float32(scale * (saa + sbb - 2.0 * sab) / denom)


def kernel(mu_a, logvar_a, mu_b, logvar_b):
    res = run_device(mu_a, logvar_a, mu_b, logvar_b, trace=False)
    return reduce_host(res.results)


# revision 6
# speedup vs baseline: 1.3586x; 1.3586x over previous
"""Gaussian distance loss kernel for 8 Trainium2 NeuronCores.

reference math (per term):
    f[i,j,d] = exp(-0.5*(mu1[i,d]-mu2[j,d])^2 / vsum) / sqrt(vsum),
    vsum = v1[i,d]+v2[j,d];  out = mean(f_aa) + mean(f_bb) - 2*mean(f_ab).

Identity used:  f = E * R * (sqrt(pi)/2)  with
    R = 1/sqrt(vsum)            (Abs_reciprocal_sqrt activation)
    E = DErf(dm*R/sqrt(2)) = (2/sqrt(pi)) * exp(-0.5*(dm*R)^2)

so each (row i, term) needs only FIVE ops on [128(d), 1024(j)] tiles:
    ACT  R  = AbsRsqrt(cv + rv_i)     (bias AP fuses the broadcast add)
    DVE  dm = cm - rm_i               (tensor_scalar, 4x mode, 327ns)
    DVE  t  = dm * R   (in-place)     (tensor_tensor, 2x mode, 594ns)
    ACT  E  = DErf(t / sqrt(2))       (in-place on t)
    DVE  stt: acc[:,col] = sum_j E*R  (scalar_tensor_tensor + accum_out;
         tensor_tensor_reduce would fit but crashes the device runtime)

This balances the two busy engines (ACT ~0.80ms, DVE ~0.79ms per pass
per core) instead of the previous DVE-bound wide-block design (DVE
1.44ms, ACT 0.99ms, 2.10ms total in CoreSim).

Abs_reciprocal_sqrt and Derivative_Erf live in DIFFERENT activation
table sets, so rows are processed in chunks of 32: all R ops of a chunk
back-to-back (one table load), then all E ops (one load) -- 24 table
switches per pass (~31us) instead of one per instruction.

Sharding: rows i split across 8 cores (128 rows per core per term);
each core holds the full transposed operands [128(d), 1024(j)].
Columns are bf16 (rounding bias cancels between vaa+vbb and -2*vab);
row scalars stay f32. Accumulation is f32 on-chip, f64 on host.
"""

import sys

for _p in ("/opt/trn_rl_repo", "/root/.axon_site/_ro/trn_rl_repo"):
    if _p not in sys.path:
        sys.path.append(_p)

import math

import ml_dtypes
import numpy as np

N = 1024
D = 128
NCORES = 8
ROWS = N // NCORES  # 128 rows per core per term
CHUNK = 32  # rows per activation-table phase
NCHUNKS = ROWS // CHUNK

O_MA, O_MB, O_VA, O_VB = 0, N, 2 * N, 3 * N
PACKED_BF_W = 4 * N
# f32 row-parameter tensor: rm_a, rv_a, rm_b, rv_b, each [D, ROWS]
O_RMA, O_RVA, O_RMB, O_RVB = 0, ROWS, 2 * ROWS, 3 * ROWS
PACKED_F_W = 4 * ROWS

NACC = 3 * ROWS  # one f32 accumulator column per (term, row)


def build_program(repeat=1, step=1):
    import concourse.bacc as bacc
    import concourse.mybir as mybir
    import concourse.tile as tile
    from concourse.alu_op_type import AluOpType

    f32 = mybir.dt.float32
    bf16 = mybir.dt.bfloat16
    Act = mybir.ActivationFunctionType
    Alu = AluOpType

    nc = bacc.Bacc("TRN2", target_bir_lowering=False, debug=False)
    inp_d = nc.dram_tensor("inp", [D, PACKED_BF_W], bf16, kind="ExternalInput")
    inpf_d = nc.dram_tensor("inpf", [D, PACKED_F_W], f32, kind="ExternalInput")
    acc_out = nc.dram_tensor("acc", [D, NACC], f32, kind="ExternalOutput")

    ISQRT2 = 1.0 / math.sqrt(2.0)

    with tile.TileContext(nc) as tc:
        with (
            tc.tile_pool(name="inputs", bufs=1) as inp,
            tc.tile_pool(name="inputsf", bufs=1) as inpf,
            tc.tile_pool(name="accp", bufs=1) as accp,
            tc.tile_pool(name="Rp", bufs=CHUNK + 4) as Rp,
            tc.tile_pool(name="Tp", bufs=CHUNK + 4) as Tp,
        ):
            big = inp.tile([D, PACKED_BF_W], bf16, tag="big")
            nc.sync.dma_start(big[:], inp_d[:])
            bigf = inpf.tile([D, PACKED_F_W], f32, tag="bigf")
            nc.sync.dma_start(bigf[:], inpf_d[:])

            cm_a = big[:, O_MA : O_MA + N]
            cm_b = big[:, O_MB : O_MB + N]
            cv_a = big[:, O_VA : O_VA + N]
            cv_b = big[:, O_VB : O_VB + N]
            rm_a = bigf[:, O_RMA : O_RMA + ROWS]
            rv_a = bigf[:, O_RVA : O_RVA + ROWS]
            rm_b = bigf[:, O_RMB : O_RMB + ROWS]
            rv_b = bigf[:, O_RVB : O_RVB + ROWS]

            acc = accp.tile([D, NACC], f32, tag="acc")

            terms = [
                (cm_a, cv_a, rm_a, rv_a),  # vaa
                (cm_b, cv_b, rm_a, rv_a),  # vab
                (cm_b, cv_b, rm_b, rv_b),  # vbb
            ] * repeat

            for ti, (cm, cv, rm, rv) in enumerate(terms):
                t0 = (ti % 3) * ROWS  # repeats overwrite the same columns
                for ch in range(NCHUNKS):
                    i0 = ch * CHUNK
                    Rts = []
                    Tts = []
                    # phase A: R = AbsRsqrt(cv + rv_i) back-to-back on ACT;
                    # DVE computes dm and t = dm*R as each R lands.
                    for r in range(0, CHUNK, step):
                        i = i0 + r
                        R = Rp.tile([D, N], bf16, tag="R")
                        nc.scalar.activation(
                            R[:], cv, Act.Abs_reciprocal_sqrt,
                            bias=rv[:, i : i + 1],
                        )
                        T = Tp.tile([D, N], bf16, tag="T")
                        nc.vector.tensor_scalar(
                            T[:], cm, rm[:, i : i + 1], None, Alu.subtract
                        )
                        nc.vector.tensor_tensor(T[:], T[:], R[:], Alu.mult)
                        Rts.append(R)
                        Tts.append(T)
                    # phase B: E = DErf(t/sqrt(2)) back-to-back on ACT;
                    # DVE reduces sum_j E*R as each E lands.
                    for k, r in enumerate(range(0, CHUNK, step)):
                        i = i0 + r
                        R, T = Rts[k], Tts[k]
                        nc.scalar.activation(
                            T[:], T[:], Act.Derivative_Erf, scale=ISQRT2
                        )
                        col = t0 + i
                        nc.vector.scalar_tensor_tensor(
                            T[:], T[:], 1.0, R[:], Alu.bypass, Alu.mult,
                            accum_out=acc[:, col : col + 1],
                        )

            nc.sync.dma_start(acc_out[:], acc[:])

    nc.compile()
    return nc


_PROGRAM_CACHE = {}


def _get_program(repeat=1, step=1):
    key = (repeat, step)
    if key not in _PROGRAM_CACHE:
        _PROGRAM_CACHE[key] = build_program(repeat, step)
    return _PROGRAM_CACHE[key]


def pack_inputs(mu_a, logvar_a, mu_b, logvar_b):
    ma_t = np.ascontiguousarray(np.asarray(mu_a).T.astype(np.float32))
    mb_t = np.ascontiguousarray(np.asarray(mu_b).T.astype(np.float32))
    va_t = np.exp(np.asarray(logvar_a).T.astype(np.float32))
    vb_t = np.exp(np.asarray(logvar_b).T.astype(np.float32))
    packed_bf = np.empty((D, PACKED_BF_W), dtype=np.float32)
    packed_bf[:, O_MA : O_MA + N] = ma_t
    packed_bf[:, O_MB : O_MB + N] = mb_t
    packed_bf[:, O_VA : O_VA + N] = va_t
    packed_bf[:, O_VB : O_VB + N] = vb_t
    packed_bf = packed_bf.astype(ml_dtypes.bfloat16)
    in_maps = []
    for c in range(NCORES):
        r0, r1 = c * ROWS, (c + 1) * ROWS
        packed_f = np.empty((D, PACKED_F_W), dtype=np.float32)
        packed_f[:, O_RMA : O_RMA + ROWS] = ma_t[:, r0:r1]
        packed_f[:, O_RVA : O_RVA + ROWS] = va_t[:, r0:r1]
        packed_f[:, O_RMB : O_RMB + ROWS] = mb_t[:, r0:r1]
        packed_f[:, O_RVB : O_RVB + ROWS] = vb_t[:, r0:r1]
        in_maps.append({"inp": packed_bf, "inpf": packed_f})
    return in_maps


def run_device(mu_a, logvar_a, mu_b, logvar_b, trace=False, repeat=1, step=1):
    from concourse.bass_utils import run_bass_kernel_spmd

    nc = _get_program(repeat, step)
    in_maps = pack_inputs(mu_a, logvar_a, mu_b, logvar_b)
    return run_bass_kernel_spmd(nc, in_maps, list(range(NCORES)), trace=trace)


def reduce_host(results):
    saa = sab = sbb = 0.0
    for r in results:
        acc = np.asarray(r["acc"], dtype=np.float64)
        saa += acc[:, 0:ROWS].sum()
        sab += acc[:, ROWS : 2 * ROWS].sum()
        sbb += acc[:, 2 * ROWS : 3 * ROWS].sum()
    denom = float(N) * N * D
    scale = math.sqrt(math.pi) / 2.0
    return np.float32(scale * (saa + sbb - 2.0 * sab) / denom)


def kernel(mu_a, logvar_a, mu_b, logvar_b):
    res = run_device(mu_a, logvar_a, mu_b, logvar_b, trace=False)
    return reduce_host(res.results)
